# revision 5
# baseline (speedup 1.0000x reference)
"""Trainium2 Bass kernel for nn_PolyAttention (16-head polynomial causal attention).

Reference math (fp32):
    q = x @ Wq.T; k = x @ Wk.T; v = x @ Wv.T        (per-head dim 128, 16 heads)
    q, k = rope(q), rope(k)                          (LRPE type-1, base 10000)
    s = (q . k)^4, causal-masked, row-normalized by max(sum, 1e-6)
    out = (s @ v normalized) @ Wo.T

Sharding: 8 cores = batch(2) x head-group(4 heads each).  Each core computes its
(b, head-group) shard end-to-end plus the Wo partial projection; the host sums
the 4 partials per batch element.

Device layout notes (per core):
  xt  [2048,2048]  x[b].T                (d on partitions, n on free dim)
  wq/wk/wv [2048, 512]   W[g_rows].T     (d x local-head-dims)
  wo  [512, 2048]        Wo[:, g_cols].T (local-c x d_out)
  qT/kT SBUF [128, 4*2048]  per-head transposed activations (dh x n), roped
  vS  SBUF [128, 16*512]    v blocks, kb-major (key-in-block x (kb, h, dh))
  scores are built transposed: sT [keys, queries] so that AV yields outT [dh, q]
  directly and the Wo matmul needs no transposes anywhere.
"""

import os
import sys

import numpy as np

if "/opt/trn_rl_repo" not in sys.path:
    sys.path.insert(0, "/opt/trn_rl_repo")

# ---------------------------------------------------------------- constants
B = 2
N = 2048
D = 2048
NH = 16
DH = 128
NHL = 4          # heads per core
HL = NHL * DH    # 512 local head dims
POLY = 4
EPS = 1e-6
LRPE_BASE = 10000.0

CH = 256         # projection n-chunk (columns of xT per step)
QB = 512         # query block
KB = 128         # key block

USE_F32R = os.environ.get("POLY_F32R", "1") == "1"


# ---------------------------------------------------------------- builder
def build_module(n=N, use_f32r=USE_F32R):
    import concourse.bacc as bacc
    import concourse.mybir as mybir
    import concourse.tile as tile

    f32 = mybir.dt.float32
    f32r = mybir.dt.float32r
    AF = mybir.ActivationFunctionType

    nc = bacc.Bacc(
        "TRN2",
        target_bir_lowering=False,
        debug=False,
        enable_asserts=False,
        num_devices=8,
    )
    mdt = f32r if use_f32r else f32

    xt_d = nc.dram_tensor("xt", [D, n], mdt, kind="ExternalInput").ap()
    wq_d = nc.dram_tensor("wq", [D, HL], mdt, kind="ExternalInput").ap()
    wk_d = nc.dram_tensor("wk", [D, HL], mdt, kind="ExternalInput").ap()
    wv_d = nc.dram_tensor("wv", [D, HL], mdt, kind="ExternalInput").ap()
    wo_d = nc.dram_tensor("wo", [HL, D], mdt, kind="ExternalInput").ap()
    cs_d = nc.dram_tensor("cs", [DH, n], f32, kind="ExternalInput").ap()
    sn_d = nc.dram_tensor("sn", [DH, n], f32, kind="ExternalInput").ap()
    mk_d = nc.dram_tensor("msk", [KB, 4 * QB], f32, kind="ExternalInput").ap()
    out_d = nc.dram_tensor("out", [n, D], f32, kind="ExternalOutput").ap()

    ndb = D // 128          # 16 d-blocks (contraction tiles)
    nch = n // CH           # projection chunks
    nqb = n // QB           # query blocks
    nkb_tot = n // KB       # key blocks

    def mm(out, lhsT, rhs, start, stop):
        nc.tensor.matmul(out, lhsT, rhs, start=start, stop=stop)

    with tile.TileContext(nc) as tc:
        from contextlib import ExitStack

        with ExitStack() as ctx:
            persist = ctx.enter_context(tc.tile_pool(name="persist", bufs=1))
            qT = persist.tile([128, NHL * n], mdt, tag="qT", name="qT")
            kT = persist.tile([128, NHL * n], mdt, tag="kT", name="kT")
            vS = persist.tile([128, nkb_tot * HL], mdt, tag="vS", name="vS")
            ones = persist.tile([128, 1], mdt, tag="ones", name="ones")
            nc.vector.memset(ones[:].bitcast(f32), 1.0)

            # ---------------- pass A: q,k projections + rope ----------------
            with tc.tile_pool(name="pA_w", bufs=1) as wpool, \
                 tc.tile_pool(name="pA_x", bufs=2) as xpool, \
                 tc.tile_pool(name="pA_t", bufs=3) as tpool, \
                 tc.tile_pool(name="pA_ps", bufs=4, space="PSUM") as pspool:
                wq_t = [wpool.tile([128, HL], mdt, tag=f"wq{i}", name=f"wq{i}") for i in range(ndb)]
                wk_t = [wpool.tile([128, HL], mdt, tag=f"wk{i}", name=f"wk{i}") for i in range(ndb)]
                for i in range(ndb):
                    nc.sync.dma_start(wq_t[i][:], wq_d[i * 128:(i + 1) * 128, :])
                    nc.sync.dma_start(wk_t[i][:], wk_d[i * 128:(i + 1) * 128, :])

                for c in range(nch):
                    c0 = c * CH
                    xt_c = [xpool.tile([128, CH], mdt, tag=f"xt{i}", name=f"xt{i}") for i in range(ndb)]
                    for i in range(ndb):
                        nc.sync.dma_start(xt_c[i][:], xt_d[i * 128:(i + 1) * 128, c0:c0 + CH])
                    cs_c = xpool.tile([128, CH], f32, tag="cs", name="cs")
                    sn_c = xpool.tile([128, CH], f32, tag="sn", name="sn")
                    nc.sync.dma_start(cs_c[:], cs_d[:, c0:c0 + CH])
                    nc.sync.dma_start(sn_c[:], sn_d[:, c0:c0 + CH])

                    for h in range(NHL):
                        for w_t, dstT in ((wq_t, qT), (wk_t, kT)):
                            ps = pspool.tile([128, CH], f32, tag="ps", name="ps")
                            for i in range(ndb):
                                mm(ps[:], w_t[i][:, h * 128:(h + 1) * 128], xt_c[i][:],
                                   start=(i == 0), stop=(i == ndb - 1))
                            # rope: dst = ps*CS + swap(ps)*SN
                            dst = dstT[:, h * n + c0: h * n + c0 + CH]
                            swp = tpool.tile([128, CH], f32, tag="swp", name="swp")
                            nc.scalar.copy(swp[0:64, :], ps[64:128, :])
                            nc.scalar.copy(swp[64:128, :], ps[0:64, :])
                            m1 = tpool.tile([128, CH], f32, tag="m1", name="m1")
                            nc.vector.tensor_mul(m1[:], ps[:], cs_c[:])
                            m2 = tpool.tile([128, CH], f32, tag="m2", name="m2")
                            nc.vector.tensor_mul(m2[:], swp[:], sn_c[:])
                            nc.vector.tensor_add(dst, m1[:], m2[:])

            # ---------------- pass B: v projection ----------------
            with tc.tile_pool(name="pB_w", bufs=1) as wpool, \
                 tc.tile_pool(name="pB_x", bufs=2) as xpool, \
                 tc.tile_pool(name="pB_ps", bufs=2, space="PSUM") as pspool:
                wv_t = [wpool.tile([128, HL], mdt, tag=f"wv{i}", name=f"wv{i}") for i in range(ndb)]
                for i in range(ndb):
                    nc.sync.dma_start(wv_t[i][:], wv_d[i * 128:(i + 1) * 128, :])
                for c in range(nch):
                    c0 = c * CH
                    xt_c = [xpool.tile([128, CH], mdt, tag=f"xt{i}", name=f"xt{i}") for i in range(ndb)]
                    for i in range(ndb):
                        nc.sync.dma_start(xt_c[i][:], xt_d[i * 128:(i + 1) * 128, c0:c0 + CH])
                    for t2 in range(CH // 128):
                        kb = c * (CH // 128) + t2
                        ps = pspool.tile([128, HL], f32, tag="psv", name="psv")
                        for i in range(ndb):
                            mm(ps[:], xt_c[i][:, t2 * 128:(t2 + 1) * 128], wv_t[i][:],
                               start=(i == 0), stop=(i == ndb - 1))
                        nc.scalar.copy(vS[:, kb * HL:(kb + 1) * HL], ps[:])

            # ---------------- pass C: attention + Wo ----------------
            with tc.tile_pool(name="pC_w", bufs=1) as wpool, \
                 tc.tile_pool(name="pC_sb", bufs=3) as spool, \
                 tc.tile_pool(name="pC_on", bufs=2) as onpool, \
                 tc.tile_pool(name="pC_fo", bufs=3) as fopool, \
                 tc.tile_pool(name="pC_ps", bufs=2, space="PSUM") as psS, \
                 tc.tile_pool(name="pC_po", bufs=2, space="PSUM") as psO, \
                 tc.tile_pool(name="pC_pd", bufs=2, space="PSUM") as psD, \
                 tc.tile_pool(name="pC_pf", bufs=2, space="PSUM") as psF:
                wo_t = [wpool.tile([128, D], mdt, tag=f"wo{i}", name=f"wo{i}") for i in range(NHL)]
                for i in range(NHL):
                    nc.sync.dma_start(wo_t[i][:], wo_d[i * 128:(i + 1) * 128, :])
                mk = wpool.tile([128, 4 * QB], f32, tag="mk", name="mk")
                nc.sync.dma_start(mk[:], mk_d[:, :])

                for qb in range(nqb):
                    onrm = [onpool.tile([128, QB], mdt, tag=f"onrm{h}", name=f"onrm{h}") for h in range(NHL)]
                    for h in range(NHL):
                        nkb = (qb + 1) * (QB // KB)
                        pso = psO.tile([128, QB], f32, tag="pso", name="pso")
                        psd = psD.tile([1, QB], f32, tag="psd", name="psd")
                        for kb in range(nkb):
                            pss = psS.tile([128, QB], f32, tag="pss", name="pss")
                            mm(pss[:], kT[:, h * n + kb * KB: h * n + (kb + 1) * KB],
                               qT[:, h * n + qb * QB: h * n + (qb + 1) * QB],
                               start=True, stop=True)
                            s2 = spool.tile([128, QB], f32, tag="s2", name="s2")
                            nc.scalar.activation(s2[:], pss[:], AF.Square)
                            rel = kb - qb * (QB // KB)
                            if rel >= 0:
                                nc.gpsimd.tensor_mul(s2[:], s2[:], mk[:, rel * QB:(rel + 1) * QB])
                            s4 = spool.tile([128, QB], mdt, tag="s4", name="s4")
                            if kb % 3 == 2:
                                nc.scalar.activation(s4[:], s2[:], AF.Square)
                            else:
                                nc.vector.tensor_mul(s4[:], s2[:], s2[:])
                            mm(pso[:], vS[:, kb * HL + h * 128: kb * HL + (h + 1) * 128],
                               s4[:], start=(kb == 0), stop=(kb == nkb - 1))
                            mm(psd[:], ones[:, 0:1], s4[:],
                               start=(kb == 0), stop=(kb == nkb - 1))
                        rec = spool.tile([1, QB], f32, tag="rec", name="rec")
                        nc.vector.tensor_scalar_max(rec[:], psd[:], EPS)
                        rec2 = spool.tile([1, QB], f32, tag="rec2", name="rec2")
                        nc.vector.reciprocal(rec2[:], rec[:])
                        rbc = spool.tile([128, QB], f32, tag="rbc", name="rbc")
                        nc.gpsimd.partition_broadcast(rbc[:], rec2[:])
                        nc.vector.tensor_mul(onrm[h][:], pso[:], rbc[:])
                    # Wo projection for this query block
                    for qt in range(QB // 128):
                        fout = fopool.tile([128, D], f32, tag="fout", name="fout")
                        for jc in range(D // 512):
                            psf = psF.tile([128, 512], f32, tag="psf", name="psf")
                            for h in range(NHL):
                                mm(psf[:], onrm[h][:, qt * 128:(qt + 1) * 128],
                                   wo_t[h][:, jc * 512:(jc + 1) * 512],
                                   start=(h == 0), stop=(h == NHL - 1))
                            if jc % 2 == 0:
                                nc.scalar.copy(fout[:, jc * 512:(jc + 1) * 512], psf[:])
                            else:
                                nc.vector.tensor_copy(fout[:, jc * 512:(jc + 1) * 512], psf[:])
                        r0 = qb * QB + qt * 128
                        nc.sync.dma_start(out_d[r0:r0 + 128, :], fout[:])

    nc.compile()
    return nc


# ---------------------------------------------------------------- host prep
def _rope_tables(n):
    half = DH // 2
    theta = LRPE_BASE ** (-np.arange(half, dtype=np.float64) * 2.0 / DH)
    pos = np.arange(n, dtype=np.float64)
    ang = np.outer(pos, theta)                       # [n, 64]
    cos = np.cos(ang).T.astype(np.float32)           # [64, n]
    sin = np.sin(ang).T.astype(np.float32)
    cs = np.concatenate([cos, cos], axis=0)          # [128, n]
    sn = np.concatenate([-sin, sin], axis=0)
    return np.ascontiguousarray(cs), np.ascontiguousarray(sn)


def _masks():
    # mask_r[kp, qi] = 1 if 128*r + kp <= qi else 0  (rel-diagonal causal tiles)
    out = np.zeros((KB, 4 * QB), dtype=np.float32)
    kp = np.arange(KB)[:, None]
    qi = np.arange(QB)[None, :]
    for r in range(4):
        out[:, r * QB:(r + 1) * QB] = (KB * r + kp <= qi).astype(np.float32)
    return out


def make_in_maps(x, Wq, Wk, Wv, Wo, n=N):
    cs, sn = _rope_tables(n)
    mk = _masks()
    xts = [np.ascontiguousarray(x[b].T) for b in range(x.shape[0])]
    in_maps = []
    for core in range(8):
        b, g = core // 4, core % 4
        rows = slice(g * HL, (g + 1) * HL)
        in_maps.append({
            "xt": xts[b],
            "wq": np.ascontiguousarray(Wq[rows, :].T),
            "wk": np.ascontiguousarray(Wk[rows, :].T),
            "wv": np.ascontiguousarray(Wv[rows, :].T),
            "wo": np.ascontiguousarray(Wo[:, rows].T),
            "cs": cs,
            "sn": sn,
            "msk": mk,
        })
    return in_maps


_NC_CACHE = {}


def _get_nc(n=N, use_f32r=USE_F32R):
    key = (n, use_f32r)
    if key not in _NC_CACHE:
        _NC_CACHE[key] = build_module(n, use_f32r)
    return _NC_CACHE[key]


def run(x, Wq, Wk, Wv, Wo, trace=False, **kw):
    from concourse.bass_utils import run_bass_kernel_spmd

    x = np.asarray(x, dtype=np.float32)
    nc = _get_nc(x.shape[1])
    in_maps = make_in_maps(x, Wq, Wk, Wv, Wo, n=x.shape[1])
    res = run_bass_kernel_spmd(nc, in_maps, core_ids=list(range(8)), trace=trace, **kw)
    b0 = res.results[0]["out"] + res.results[1]["out"] + res.results[2]["out"] + res.results[3]["out"]
    b1 = res.results[4]["out"] + res.results[5]["out"] + res.results[6]["out"] + res.results[7]["out"]
    out = np.stack([b0, b1]).astype(np.float32)
    return out, res


def kernel(x, Wq, Wk, Wv, Wo):
    out, _ = run(
        np.asarray(x, np.float32),
        np.asarray(Wq, np.float32),
        np.asarray(Wk, np.float32),
        np.asarray(Wv, np.float32),
        np.asarray(Wo, np.float32),
    )
    return out


# revision 16
# speedup vs baseline: 9.4221x; 9.4221x over previous
"""Trainium2 Bass kernel for nn_PolyAttention (16-head polynomial causal attention).

Reference math (fp32):
    q = x @ Wq.T; k = x @ Wk.T; v = x @ Wv.T        (per-head dim 128, 16 heads)
    q, k = rope(q), rope(k)                          (LRPE type-1, base 10000)
    s = (q . k)^4, causal-masked, row-normalized by max(sum, 1e-6)
    out = (s @ v normalized) @ Wo.T

Sharding: 8 cores = batch(2) x head-group(4 heads each).  Each core computes its
(b, head-group) shard end-to-end plus the Wo partial projection; the host sums
the 4 partials per batch element.

Device layout notes (per core):
  xt  [2048,2048]  x[b].T                (d on partitions, n on free dim)
  wq/wk/wv [2048, 512]   W[g_rows].T     (d x local-head-dims)
  wo  [512, 2048]        Wo[:, g_cols].T (local-c x d_out)
  qT/kT SBUF [128, 4*2048]  per-head transposed activations (dh x n), roped
  vS  SBUF [128, 16*512]    v blocks, kb-major (key-in-block x (kb, h, dh))
  scores are built transposed: sT [keys, queries] so that AV yields outT [dh, q]
  directly and the Wo matmul needs no transposes anywhere.
"""

import os
import sys

import numpy as np

if "/opt/trn_rl_repo" not in sys.path:
    sys.path.insert(0, "/opt/trn_rl_repo")

# ---------------------------------------------------------------- constants
B = 2
N = 2048
D = 2048
NH = 16
DH = 128
NHL = 4          # heads per core
HL = NHL * DH    # 512 local head dims
POLY = 4
EPS = 1e-6
LRPE_BASE = 10000.0

CH = 256         # projection n-chunk (columns of xT per step)
QB = 512         # query block
KB = 128         # key block

USE_F32R = os.environ.get("POLY_F32R", "1") == "1"


# ---------------------------------------------------------------- builder
def build_module(n=N, use_f32r=USE_F32R):
    import concourse.bacc as bacc
    import concourse.mybir as mybir
    import concourse.tile as tile

    f32 = mybir.dt.float32
    f32r = mybir.dt.float32r
    AF = mybir.ActivationFunctionType

    nc = bacc.Bacc(
        "TRN2",
        target_bir_lowering=False,
        debug=False,
        enable_asserts=False,
        num_devices=8,
    )
    mdt = f32r if use_f32r else f32

    xt_d = nc.dram_tensor("xt", [D, n], mdt, kind="ExternalInput").ap()
    wq_d = nc.dram_tensor("wq", [D, HL], mdt, kind="ExternalInput").ap()
    wk_d = nc.dram_tensor("wk", [D, HL], mdt, kind="ExternalInput").ap()
    wv_d = nc.dram_tensor("wv", [D, HL], mdt, kind="ExternalInput").ap()
    wo_d = nc.dram_tensor("wo", [HL, D], mdt, kind="ExternalInput").ap()
    cs_d = nc.dram_tensor("cs", [DH, n], f32, kind="ExternalInput").ap()
    sn_d = nc.dram_tensor("sn", [DH, n], f32, kind="ExternalInput").ap()
    mk_d = nc.dram_tensor("msk", [KB, 3 * KB], f32, kind="ExternalInput").ap()
    out_d = nc.dram_tensor("out", [n, D], f32, kind="ExternalOutput").ap()

    ndb = D // 128          # 16 d-blocks (contraction tiles)
    nch = n // CH           # projection chunks
    nqb = n // QB           # query blocks
    nkb_tot = n // KB       # key blocks

    def mm(out, lhsT, rhs, start, stop):
        nc.tensor.matmul(out, lhsT, rhs, start=start, stop=stop)

    with tile.TileContext(nc) as tc:
        from contextlib import ExitStack

        with ExitStack() as ctx:
            persist = ctx.enter_context(tc.tile_pool(name="persist", bufs=1))
            qT = persist.tile([128, NHL * n], mdt, tag="qT", name="qT")
            kT = persist.tile([128, NHL * n], mdt, tag="kT", name="kT")
            vS = persist.tile([128, nkb_tot * HL], mdt, tag="vS", name="vS")
            ones = persist.tile([128, 1], mdt, tag="ones", name="ones")
            nc.vector.memset(ones[:].bitcast(f32), 1.0)

            # ---------------- pass A: q,k projections + rope ----------------
            with tc.tile_pool(name="pA_w", bufs=1) as wpool, \
                 tc.tile_pool(name="pA_x", bufs=2) as xpool, \
                 tc.tile_pool(name="pA_t", bufs=3) as tpool, \
                 tc.tile_pool(name="pA_ps", bufs=6, space="PSUM") as pspool:
                wq_t = [wpool.tile([128, HL], mdt, tag=f"wq{i}", name=f"wq{i}") for i in range(ndb)]
                wk_t = [wpool.tile([128, HL], mdt, tag=f"wk{i}", name=f"wk{i}") for i in range(ndb)]

                for c in range(nch):
                    c0 = c * CH
                    xt_c = [xpool.tile([128, CH], mdt, tag=f"xt{i}", name=f"xt{i}") for i in range(ndb)]
                    cs_c = xpool.tile([128, CH], f32, tag="cs", name="cs")
                    sn_c = xpool.tile([128, CH], f32, tag="sn", name="sn")
                    nc.sync.dma_start(cs_c[:], cs_d[:, c0:c0 + CH])
                    nc.sync.dma_start(sn_c[:], sn_d[:, c0:c0 + CH])
                    # interleave weight loads with the first x chunk so the
                    # first accumulation chain starts after ~3 tiles, not 10MB
                    for i in range(ndb):
                        nc.sync.dma_start(xt_c[i][:], xt_d[i * 128:(i + 1) * 128, c0:c0 + CH])
                        if c == 0:
                            nc.sync.dma_start(wq_t[i][:], wq_d[i * 128:(i + 1) * 128, :])
                            nc.sync.dma_start(wk_t[i][:], wk_d[i * 128:(i + 1) * 128, :])

                    for h in range(NHL):
                        for w_t, dstT in ((wq_t, qT), (wk_t, kT)):
                            ps = pspool.tile([128, CH], f32, tag="ps", name="ps")
                            for i in range(ndb):
                                mm(ps[:], w_t[i][:, h * 128:(h + 1) * 128], xt_c[i][:],
                                   start=(i == 0), stop=(i == ndb - 1))
                            # rope: dst = ps*CS + swap(ps)*SN
                            dst = dstT[:, h * n + c0: h * n + c0 + CH]
                            swp = tpool.tile([128, CH], f32, tag="swp", name="swp")
                            nc.scalar.copy(swp[0:64, :], ps[64:128, :])
                            nc.scalar.copy(swp[64:128, :], ps[0:64, :])
                            m1 = tpool.tile([128, CH], f32, tag="m1", name="m1")
                            nc.vector.tensor_mul(m1[:], ps[:], cs_c[:])
                            m2 = tpool.tile([128, CH], f32, tag="m2", name="m2")
                            nc.gpsimd.tensor_mul(m2[:], swp[:], sn_c[:])
                            nc.vector.tensor_add(dst, m1[:], m2[:])

            # ---------------- pass B: v projection ----------------
            with tc.tile_pool(name="pB_w", bufs=1) as wpool, \
                 tc.tile_pool(name="pB_x", bufs=2) as xpool, \
                 tc.tile_pool(name="pB_ps", bufs=4, space="PSUM") as pspool:
                wv_t = [wpool.tile([128, HL], mdt, tag=f"wv{i}", name=f"wv{i}") for i in range(ndb)]
                for c in range(nch):
                    c0 = c * CH
                    xt_c = [xpool.tile([128, CH], mdt, tag=f"xt{i}", name=f"xt{i}") for i in range(ndb)]
                    for i in range(ndb):
                        nc.sync.dma_start(xt_c[i][:], xt_d[i * 128:(i + 1) * 128, c0:c0 + CH])
                        if c == 0:
                            nc.sync.dma_start(wv_t[i][:], wv_d[i * 128:(i + 1) * 128, :])
                    for t2 in range(CH // 128):
                        kb = c * (CH // 128) + t2
                        ps = pspool.tile([128, HL], f32, tag="psv", name="psv")
                        for i in range(ndb):
                            mm(ps[:], xt_c[i][:, t2 * 128:(t2 + 1) * 128], wv_t[i][:],
                               start=(i == 0), stop=(i == ndb - 1))
                        nc.scalar.copy(vS[:, kb * HL:(kb + 1) * HL], ps[:])

            # ---------------- pass C: attention + Wo ----------------
            with tc.tile_pool(name="pC_w", bufs=1) as wpool, \
                 tc.tile_pool(name="pC_sb", bufs=4) as spool, \
                 tc.tile_pool(name="pC_on", bufs=2) as onpool, \
                 tc.tile_pool(name="pC_fo", bufs=2) as fopool, \
                 tc.tile_pool(name="pC_ps", bufs=4, space="PSUM") as psS, \
                 tc.tile_pool(name="pC_po", bufs=2, space="PSUM") as psO, \
                 tc.tile_pool(name="pC_pd", bufs=1, space="PSUM") as psD, \
                 tc.tile_pool(name="pC_pf", bufs=1, space="PSUM") as psF:
                wo_t = [wpool.tile([128, D], mdt, tag=f"wo{i}", name=f"wo{i}") for i in range(NHL)]
                mk = wpool.tile([128, 3 * KB], f32, tag="mk", name="mk")
                nc.sync.dma_start(mk[:], mk_d[:, :])
                wo_emitted = False

                for qb in range(nqb):
                    onrm = [onpool.tile([128, QB], mdt, tag=f"onrm{h}", name=f"onrm{h}") for h in range(NHL)]
                    for h in range(NHL):
                        nkb = (qb + 1) * (QB // KB)
                        pso = psO.tile([128, QB], f32, tag="pso", name="pso")
                        psd = psD.tile([1, QB], f32, tag="psd", name="psd")
                        for kb in range(nkb):
                            rel = kb - qb * (QB // KB)
                            # band blocks: only queries >= cr can attend to this
                            # key block; skip the dead columns entirely
                            cr = 0 if rel < 0 else min(KB * rel, 2 * KB)
                            w = QB - cr
                            pss = psS.tile([128, QB], f32, tag="pss", name="pss")
                            mm(pss[:, cr:], kT[:, h * n + kb * KB: h * n + (kb + 1) * KB],
                               qT[:, h * n + qb * QB + cr: h * n + (qb + 1) * QB],
                               start=True, stop=True)
                            s2 = spool.tile([128, QB], f32, tag="s2", name="s2")
                            nc.scalar.activation(s2[:, cr:], pss[:, cr:], AF.Square)
                            if rel >= 0:
                                if rel < 3:
                                    nc.gpsimd.tensor_mul(s2[:, KB * rel:KB * (rel + 1)],
                                                         s2[:, KB * rel:KB * (rel + 1)],
                                                         mk[:, 0:KB])
                                else:
                                    nc.gpsimd.tensor_mul(s2[:, 2 * KB:], s2[:, 2 * KB:],
                                                         mk[:, KB:3 * KB])
                            s4 = spool.tile([128, QB], mdt, tag="s4", name="s4")
                            if kb % 4 == 2:
                                nc.gpsimd.tensor_mul(s4[:, cr:], s2[:, cr:], s2[:, cr:])
                            else:
                                nc.vector.tensor_mul(s4[:, cr:], s2[:, cr:], s2[:, cr:])
                            mm(pso[:, cr:], vS[:, kb * HL + h * 128: kb * HL + (h + 1) * 128],
                               s4[:, cr:], start=(kb == 0), stop=(kb == nkb - 1))
                            mm(psd[0:1, cr:], ones[:, 0:1], s4[:, cr:],
                               start=(kb == 0), stop=(kb == nkb - 1))
                        if qb == 0 and h == 0 and not wo_emitted:
                            # prefetch Wo during the first head's attention
                            wo_emitted = True
                            for i in range(NHL):
                                nc.sync.dma_start(wo_t[i][:], wo_d[i * 128:(i + 1) * 128, :])
                        rec = spool.tile([1, QB], f32, tag="rec", name="rec")
                        nc.vector.tensor_scalar_max(rec[:], psd[0:1, :], EPS)
                        rec2 = spool.tile([1, QB], f32, tag="rec2", name="rec2")
                        nc.vector.reciprocal(rec2[:], rec[:])
                        rbc = spool.tile([128, QB], f32, tag="rbc", name="rbc")
                        nc.gpsimd.partition_broadcast(rbc[:], rec2[:])
                        nc.vector.tensor_mul(onrm[h][:], pso[:], rbc[:])
                    # Wo projection for this query block
                    for qt in range(QB // 128):
                        fout = fopool.tile([128, D], f32, tag="fout", name="fout")
                        for jc in range(D // 512):
                            psf = psF.tile([128, 512], f32, tag="psf", name="psf")
                            for h in range(NHL):
                                mm(psf[:], onrm[h][:, qt * 128:(qt + 1) * 128],
                                   wo_t[h][:, jc * 512:(jc + 1) * 512],
                                   start=(h == 0), stop=(h == NHL - 1))
                            if jc % 2 == 0:
                                nc.scalar.copy(fout[:, jc * 512:(jc + 1) * 512], psf[:])
                            else:
                                nc.vector.tensor_copy(fout[:, jc * 512:(jc + 1) * 512], psf[:])
                        r0 = qb * QB + qt * 128
                        nc.sync.dma_start(out_d[r0:r0 + 128, :], fout[:])

    nc.compile()
    return nc


# ---------------------------------------------------------------- host prep
def _rope_tables(n):
    half = DH // 2
    theta = LRPE_BASE ** (-np.arange(half, dtype=np.float64) * 2.0 / DH)
    pos = np.arange(n, dtype=np.float64)
    ang = np.outer(pos, theta)                       # [n, 64]
    cos = np.cos(ang).T.astype(np.float32)           # [64, n]
    sin = np.sin(ang).T.astype(np.float32)
    cs = np.concatenate([cos, cos], axis=0)          # [128, n]
    sn = np.concatenate([-sin, sin], axis=0)
    return np.ascontiguousarray(cs), np.ascontiguousarray(sn)


def _masks():
    # cols 0:128   = tri mask (kp <= j), applied to the diagonal 128-col strip
    #                of rel-0/1/2 band blocks
    # cols 128:384 = rel-3 mask over its 256 computed cols (kp <= j - 128)
    out = np.zeros((KB, 3 * KB), dtype=np.float32)
    kp = np.arange(KB)[:, None]
    j1 = np.arange(KB)[None, :]
    j2 = np.arange(2 * KB)[None, :]
    out[:, :KB] = (kp <= j1).astype(np.float32)
    out[:, KB:] = (kp <= j2 - KB).astype(np.float32)
    return out


def make_in_maps(x, Wq, Wk, Wv, Wo, n=N):
    cs, sn = _rope_tables(n)
    mk = _masks()
    xts = [np.ascontiguousarray(x[b].T) for b in range(x.shape[0])]
    in_maps = []
    for core in range(8):
        b, g = core // 4, core % 4
        rows = slice(g * HL, (g + 1) * HL)
        in_maps.append({
            "xt": xts[b],
            "wq": np.ascontiguousarray(Wq[rows, :].T),
            "wk": np.ascontiguousarray(Wk[rows, :].T),
            "wv": np.ascontiguousarray(Wv[rows, :].T),
            "wo": np.ascontiguousarray(Wo[:, rows].T),
            "cs": cs,
            "sn": sn,
            "msk": mk,
        })
    return in_maps


_NC_CACHE = {}


def _get_nc(n=N, use_f32r=USE_F32R):
    key = (n, use_f32r)
    if key not in _NC_CACHE:
        _NC_CACHE[key] = build_module(n, use_f32r)
    return _NC_CACHE[key]


def run(x, Wq, Wk, Wv, Wo, trace=False, **kw):
    from concourse.bass_utils import run_bass_kernel_spmd

    x = np.asarray(x, dtype=np.float32)
    nc = _get_nc(x.shape[1])
    in_maps = make_in_maps(x, Wq, Wk, Wv, Wo, n=x.shape[1])
    res = run_bass_kernel_spmd(nc, in_maps, core_ids=list(range(8)), trace=trace, **kw)
    b0 = res.results[0]["out"] + res.results[1]["out"] + res.results[2]["out"] + res.results[3]["out"]
    b1 = res.results[4]["out"] + res.results[5]["out"] + res.results[6]["out"] + res.results[7]["out"]
    out = np.stack([b0, b1]).astype(np.float32)
    return out, res


def kernel(x, Wq, Wk, Wv, Wo):
    out, _ = run(
        np.asarray(x, np.float32),
        np.asarray(Wq, np.float32),
        np.asarray(Wk, np.float32),
        np.asarray(Wv, np.float32),
        np.asarray(Wo, np.float32),
    )
    return out
